# revision 4
# baseline (speedup 1.0000x reference)
"""AttFlowLayer (BiDAF attention-flow) Trainium2 kernel, data-parallel over batch.

Problem: B=8, Lc=2048, Lq=1024, D=256 (all fp32).
  S[b,i,j] = c_i.w_c + q_j.w_q + sum_d c[b,i,d]*w_m[d]*q[j,d]
  P = softmax(S, axis=i);  A[b,j,i] = P[b,i,j] * qmask[j]
  H[b,j,:] = sum_i A[b,j,i] * c[b,i,:]
  G = [c, c * colsum(A)]   (colsum over j)

Key algebra used on-device (per batch element = per core):
  * the q_j.w_q term is constant over i, so it cancels in the softmax -> dropped.
  * c_i.w_c folds into the S matmul:  S'[i,j] = sum_d Ct[d,i] * (q[j,d]*w_m[d] + w_c[d])
  * softmax normalization is deferred: E = exp(S'), colsumE via a ones-column
    appended to C in the H matmul, then H and colsum(A) are scaled by
    r[j] = qmask[j]/colsumE[j].

One batch element per NeuronCore; no collectives.
"""

import sys

if "/opt/trn_rl_repo" not in sys.path:
    sys.path.insert(0, "/opt/trn_rl_repo")

from contextlib import ExitStack

import numpy as np

import concourse.tile as tile
from concourse import bacc, mybir
from concourse.bass_utils import run_bass_kernel_spmd
from concourse.masks import make_identity

LC, LQ, D, B = 2048, 1024, 256, 8
P = 128
NT_I, NT_J, ND = LC // P, LQ // P, D // P  # 16, 8, 2
F32, BF16 = mybir.dt.float32, mybir.dt.bfloat16
AL = mybir.AluOpType
AF = mybir.ActivationFunctionType
AX = mybir.AxisListType

_CACHED_NC = None


def _program(tc, ctx_d, q_d, w_d, g_d, h_d, stage=99):
    nc = tc.nc
    ctx_re = ctx_d.rearrange("(t p) d -> p t d", p=P)  # [128, 16, 256]
    q_re = q_d.rearrange("(t p) d -> p t d", p=P)  # [128, 8, 256]
    g_re = g_d.rearrange("(t p) e -> p t e", p=P)  # [128, 16, 512]
    h_re = h_d.rearrange("(t p) d -> p t d", p=P)  # [128, 8, 256]

    with ExitStack() as ex:
        const = ex.enter_context(tc.tile_pool(name="const", bufs=1))
        sb = ex.enter_context(tc.tile_pool(name="sb", bufs=1))
        work = ex.enter_context(tc.tile_pool(name="work", bufs=2))
        ps_s = ex.enter_context(tc.tile_pool(name="ps_s", bufs=4, space="PSUM"))
        ps_h = ex.enter_context(tc.tile_pool(name="ps_h", bufs=2, space="PSUM"))
        ps_t = ex.enter_context(tc.tile_pool(name="ps_t", bufs=2, space="PSUM"))
        dram = ex.enter_context(tc.tile_pool(name="dram", bufs=1, space="DRAM"))

        ident = const.tile([P, P], F32, tag="ident", name="ident")
        make_identity(nc, ident)
        ones_row = const.tile([1, P], F32, tag="ones_row", name="ones_row")
        nc.vector.memset(ones_row, 1.0)

        # w_c / w_m in column layout [128, ND] (partition = d mod 128)
        wc_col = const.tile([P, ND], F32, tag="wc", name="wc_col")
        wm_col = const.tile([P, ND], F32, tag="wm", name="wm_col")
        with nc.allow_non_contiguous_dma(reason="tiny 1KB const load"):
            nc.sync.dma_start(wc_col, w_d[0:D].rearrange("(t p) -> p t", p=P))
            nc.sync.dma_start(wm_col, w_d[2 * D : 3 * D].rearrange("(t p) -> p t", p=P))

        # ---- loads ----
        with nc.named_scope("load"):
            q_sb = sb.tile([P, NT_J, D], F32, tag="q_sb", name="q_sb")
            nc.sync.dma_start(q_sb, q_re)
            c_sb = sb.tile([P, NT_I, D], F32, tag="c_sb", name="c_sb")
            CH = 4
            for ch in range(NT_I // CH):
                sl = slice(ch * CH, (ch + 1) * CH)
                nc.sync.dma_start(c_sb[:, sl, :], ctx_re[:, sl, :])
                # G[:, 0:D] = context (pass-through)
                nc.sync.dma_start(g_re[:, sl, 0:D], c_sb[:, sl, :])

        if stage < 2:
            return
        # ---- transposes ----
        # qpp[dt][d_loc, j] = q[j, dt*128+d_loc] * w_m[...] + w_c[...]   (bf16)
        with nc.named_scope("tpose"):
            qpp = [
                const.tile([P, LQ], BF16, tag=f"qpp{dt}", name=f"qpp{dt}")
                for dt in range(ND)
            ]
            for m in range(NT_J):
                for dt in range(ND):
                    pt = ps_t.tile([P, P], F32, tag="pt", name="pt")
                    nc.tensor.transpose(pt, q_sb[:, m, dt * P : (dt + 1) * P], ident)
                    nc.vector.tensor_scalar(
                        qpp[dt][:, m * P : (m + 1) * P],
                        pt,
                        wm_col[:, dt : dt + 1],
                        wc_col[:, dt : dt + 1],
                        AL.mult,
                        AL.add,
                    )
            # ct[dt][d_loc, i] = c[i, dt*128+d_loc]   (bf16)
            ct = [
                const.tile([P, LC], BF16, tag=f"ct{dt}", name=f"ct{dt}")
                for dt in range(ND)
            ]
            for i in range(NT_I):
                for dt in range(ND):
                    pt = ps_t.tile([P, P], F32, tag="pt", name="pt")
                    nc.tensor.transpose(pt, c_sb[:, i, dt * P : (dt + 1) * P], ident)
                    nc.vector.tensor_copy(ct[dt][:, i * P : (i + 1) * P], pt)

        if stage < 3:
            return
        # C in bf16 with a ones column appended (rhs of the H matmul)
        cpp = [
            sb.tile([P, D + 1], BF16, tag=f"cpp{i}", name=f"cpp{i}")
            for i in range(NT_I)
        ]
        for i in range(NT_I):
            nc.scalar.copy(cpp[i][:, 0:D], c_sb[:, i, :])
            nc.gpsimd.memset(cpp[i][:, D : D + 1], 1.0)

        # qmask[j] = (sum_d q[j,d]) != 0
        qmask = const.tile([P, NT_J], F32, tag="qmask", name="qmask")
        for m in range(NT_J):
            qs = work.tile([P, 1], F32, tag="qs", name="qs")
            nc.vector.tensor_reduce(qs, q_sb[:, m, :], AX.X, AL.add)
            nc.vector.tensor_scalar(
                qmask[:, m : m + 1], qs, 0.0, None, AL.not_equal
            )

        if stage < 4:
            return
        # ---- S matmul + exp ----
        # E[i][:, j] = exp(sum_dt ct[dt][:,i-chunk].T @ qpp[dt][:, j-chunk])
        e_sb = [
            sb.tile([P, LQ], BF16, tag=f"e{i}", name=f"e{i}") for i in range(NT_I)
        ]
        with nc.named_scope("smm"):
            for i in range(NT_I):
                pss = [
                    ps_s.tile([P, 512], F32, tag="ps_s", name="pss")
                    for _ in range(2)
                ]
                for dt in range(ND):
                    for jc in range(2):
                        nc.tensor.matmul(
                            pss[jc],
                            ct[dt][:, i * P : (i + 1) * P],
                            qpp[dt][:, jc * 512 : (jc + 1) * 512],
                            start=(dt == 0),
                            stop=(dt == ND - 1),
                        )
                for jc in range(2):
                    nc.scalar.activation(
                        e_sb[i][:, jc * 512 : (jc + 1) * 512], pss[jc], AF.Exp
                    )

        if stage < 5:
            return
        # ---- H matmul (with colsumE in column D) + normalize ----
        r_all = const.tile([P, NT_J], F32, tag="r_all", name="r_all")
        with nc.named_scope("hmm"):
            for m in range(NT_J):
                ph = ps_h.tile([P, D + 1], F32, tag="ps_h", name="ph")
                for i in range(NT_I):
                    nc.tensor.matmul(
                        ph,
                        e_sb[i][:, m * P : (m + 1) * P],
                        cpp[i],
                        start=(i == 0),
                        stop=(i == NT_I - 1),
                    )
                rec = work.tile([P, 1], F32, tag="rec", name="rec")
                nc.vector.reciprocal(rec, ph[:, D : D + 1])
                nc.vector.tensor_tensor(
                    r_all[:, m : m + 1], rec, qmask[:, m : m + 1], AL.mult
                )
                hs = work.tile([P, D], F32, tag="hs", name="hs")
                nc.vector.tensor_scalar(hs, ph[:, 0:D], r_all[:, m : m + 1], None, AL.mult)
                nc.sync.dma_start(h_re[:, m, :], hs)

        if stage < 6:
            return
        # ---- r -> row layout -> broadcast over partitions ----
        with nc.named_scope("tail"):
            r_dram = dram.tile([P, NT_J], F32, tag="r_dram", name="r_dram")
            nc.sync.dma_start(r_dram, r_all)
            r_row = const.tile([1, LQ], F32, tag="r_row", name="r_row")
            with nc.allow_non_contiguous_dma(reason="tiny 4KB gather"):
                nc.sync.dma_start(
                    r_row.rearrange("o (t p) -> o t p", p=P),
                    r_dram.rearrange("p t -> t p")[None, :, :],
                )
            rb = const.tile([P, LQ], BF16, tag="rb", name="rb")
            for jc in range(2):
                pr = ps_s.tile([P, 512], F32, tag="ps_s", name="pr")
                nc.tensor.matmul(
                    pr,
                    ones_row,
                    r_row[0:1, jc * 512 : (jc + 1) * 512],
                    start=True,
                    stop=True,
                )
                nc.vector.tensor_copy(rb[:, jc * 512 : (jc + 1) * 512], pr)

            if stage < 7:
                return
            # colsum_P[i] = sum_j E[i,j]*r[j];  G[:, D:2D] = c * colsum_P
            for i in range(NT_I):
                scr = work.tile([P, LQ], BF16, tag="scr", name="scr")
                colp = work.tile([P, 1], F32, tag="colp", name="colp")
                nc.vector.tensor_tensor(scr, e_sb[i], rb, AL.mult)
                nc.vector.tensor_reduce(colp, scr, AX.X, AL.add)
                ga = work.tile([P, D], F32, tag="ga", name="ga")
                nc.vector.tensor_scalar(ga, c_sb[:, i, :], colp, None, AL.mult)
                nc.sync.dma_start(g_re[:, i, D : 2 * D], ga)


def _build(stage=99):
    nc = bacc.Bacc("TRN2", target_bir_lowering=False, debug=False, num_devices=B)
    ctx_d = nc.dram_tensor("ctx", [LC, D], F32, kind="ExternalInput").ap()
    q_d = nc.dram_tensor("q", [LQ, D], F32, kind="ExternalInput").ap()
    w_d = nc.dram_tensor("w", [3 * D], F32, kind="ExternalInput").ap()
    g_d = nc.dram_tensor("g", [LC, 2 * D], F32, kind="ExternalOutput").ap()
    h_d = nc.dram_tensor("h", [LQ, D], F32, kind="ExternalOutput").ap()
    with tile.TileContext(nc) as tc:
        _program(tc, ctx_d, q_d, w_d, g_d, h_d, stage=stage)
    nc.compile()
    return nc


def _get_nc():
    global _CACHED_NC
    if _CACHED_NC is None:
        _CACHED_NC = _build()
    return _CACHED_NC


def _make_in_maps(context, query, w_alpha):
    context = np.asarray(context, dtype=np.float32)
    query = np.ascontiguousarray(np.asarray(query, dtype=np.float32))
    w_alpha = np.ascontiguousarray(np.asarray(w_alpha, dtype=np.float32))
    return [
        {"ctx": np.ascontiguousarray(context[b]), "q": query, "w": w_alpha}
        for b in range(B)
    ]


def _run_spmd(in_maps, **kw):
    return run_bass_kernel_spmd(_get_nc(), in_maps, core_ids=list(range(B)), **kw)


def kernel(context, query, w_alpha):
    res = _run_spmd(_make_in_maps(context, query, w_alpha))
    G = np.stack([res.results[b]["g"] for b in range(B)])
    H = np.stack([res.results[b]["h"] for b in range(B)])
    return (G, H)
